# revision 1
# baseline (speedup 1.0000x reference)
"""DCGRU cell Trainium2 kernel.

Math (per batch i):
  xs = [input, state]                                  [N, 66]
  aggr[j] = S[j] @ xs          (J=4 supports)          [N, 66]
  r = sigmoid(sum_j aggr[j] @ Wr[j] + br)              [N, 64]
  u = sigmoid(sum_j aggr[j] @ Wu[j] + bu)
  xc = [input, r*state]
  c = tanh(sum_j (S[j] @ xc) @ Wc[j] + bc)
  out = u*state + (1-u)*c

Sharding: data-parallel over batch, 8 batches per core on 8 cores.
supports/weights replicated. No collectives.

Device kernel layout (per core, Bc=8):
  - Host pre-transposes supports: ST[j] = S[j].T ([m, k], m = contraction),
    cast fp16 -> stationary matmul operands are contiguous row-block slices.
  - XS packed [m=2048, (i=8, f=66)] fp16: moving operand, SBUF resident.
  - Big matmul accumulates aggr[j] = [k, (i,f)] in PSUM over 16 m-blocks;
    528-col batches split 264+264 across two banks; kb/h-major mb-minor
    order so each bank's drain overlaps the next bank's matmuls.
  - PE-transpose [128, 66] slices -> aggT[i][j] [66, 256], W-projection
    (contract 66, fp32r, accumulate over j in PSUM), bias+activation on
    ScalarE -> ru.T [128 = (r|u), k] per batch.
  - Phase 2 identical with xc; c.T overwrites the dead r.T rows.
  - GRU combine on DVE in [64, N] layout; host undoes the final transpose.
"""

import sys

if '/opt/trn_rl_repo' not in sys.path:
    sys.path.insert(0, '/opt/trn_rl_repo')

import numpy as np

B, N, IN, OUT, J = 64, 2048, 2, 64, 4
NCORES = 8
BC = B // NCORES            # 8 batches per core
F = IN + OUT                # 66
CB = BC * F                 # 528 moving columns
P = 128
HALF = CB // 2              # 264 (psum bank split)
NMB = N // P                # 16 m blocks
NKB = N // P                # 16 k blocks
KBG = 2                     # k blocks per psum group
NG = NKB // KBG             # 8 groups
MBQ = 8                     # m blocks per ST dma

MM16 = True                 # fp16 big-matmul operands (vs float32r)

_CACHE = {}


def _build_module():
    import concourse.tile as tile
    import concourse.mybir as mybir
    from concourse import bacc
    from concourse.masks import make_identity

    f32 = mybir.dt.float32
    f32r = mybir.dt.float32r
    mmdt = mybir.dt.float16 if MM16 else f32r
    AF = mybir.ActivationFunctionType

    nc = bacc.Bacc("TRN2", target_bir_lowering=False, debug=False,
                   num_devices=1)

    st_d = nc.dram_tensor("st", [J, N, N], mmdt, kind="ExternalInput").ap()
    xs_d = nc.dram_tensor("xs", [N, CB], mmdt, kind="ExternalInput").ap()
    xin_d = nc.dram_tensor("xin", [N, BC * IN], f32, kind="ExternalInput").ap()
    stT_d = nc.dram_tensor("stT", [BC, OUT, N], f32, kind="ExternalInput").ap()
    wru_d = nc.dram_tensor("wru", [J, F, 2 * OUT], mmdt, kind="ExternalInput").ap()
    wc_d = nc.dram_tensor("wc", [J, F, OUT], mmdt, kind="ExternalInput").ap()
    bru_d = nc.dram_tensor("bru", [2 * OUT, 1], f32, kind="ExternalInput").ap()
    bc_d = nc.dram_tensor("bc", [OUT, 1], f32, kind="ExternalInput").ap()
    outT_d = nc.dram_tensor("outT", [BC, OUT, N], f32, kind="ExternalOutput").ap()

    with tile.TileContext(nc) as tc:
        with tc.tile_pool(name="const", bufs=1) as const_pool, \
             tc.tile_pool(name="xs", bufs=18) as xs_pool, \
             tc.tile_pool(name="xin", bufs=16) as xin_pool, \
             tc.tile_pool(name="ruT", bufs=BC) as ruT_pool, \
             tc.tile_pool(name="stT", bufs=2) as stT_pool:

            ident = const_pool.tile([P, P], mmdt, tag="ident")
            make_identity(nc, ident[:])

            wru_t = []
            wc_t = []
            for j in range(J):
                w1 = const_pool.tile([F, 2 * OUT], mmdt, tag=f"wru{j}")
                nc.sync.dma_start(w1[:], wru_d[j])
                wru_t.append(w1)
                w2 = const_pool.tile([F, OUT], mmdt, tag=f"wc{j}")
                nc.sync.dma_start(w2[:], wc_d[j])
                wc_t.append(w2)
            bru_t = const_pool.tile([2 * OUT, 1], f32, tag="bru")
            nc.sync.dma_start(bru_t[:], bru_d[:])
            bc_t = const_pool.tile([OUT, 1], f32, tag="bc")
            nc.sync.dma_start(bc_t[:], bc_d[:])

            xs_tiles = []
            for mb in range(NMB):
                t = xs_pool.tile([P, CB], mmdt, tag="xs")
                nc.sync.dma_start(t[:], xs_d[mb * P:(mb + 1) * P, :])
                xs_tiles.append(t)
            xin_tiles = []
            for mb in range(NMB):
                t = xin_pool.tile([P, BC * IN], f32, tag="xin")
                nc.sync.dma_start(t[:], xin_d[mb * P:(mb + 1) * P, :])
                xin_tiles.append(t)

            ruT_tiles = [ruT_pool.tile([P, N], f32, tag="ruT", name=f"ruT{i}")
                         for i in range(BC)]

            def big_phase(x_tiles, w_tiles, out_rows, bias_t, act_fn,
                          out_slice_fn):
                """One graph-conv pass + projection + activation.

                out_slice_fn(i, k0, width) -> SBUF AP [out_rows, width]
                receiving act(proj + bias) for batch i, k cols [k0, k0+w).
                """
                with tc.tile_pool(name="stst", bufs=3) as st_pool, \
                     tc.tile_pool(name="agg", bufs=16) as agg_pool, \
                     tc.tile_pool(name="aggT", bufs=8) as aggT_pool, \
                     tc.tile_pool(name="aggps", bufs=4, space="PSUM") as agg_ps_pool, \
                     tc.tile_pool(name="tpps", bufs=4, space="PSUM") as tp_ps_pool:
                    for g in range(NG):
                        k0 = g * KBG * P        # 256-aligned k offset
                        agg_sb = {}
                        for j in range(J):
                            st_ts = []
                            for mq in range(NMB // MBQ):
                                st_t = st_pool.tile([P, MBQ, KBG * P], mmdt,
                                                    tag="st")
                                src = st_d[j, mq * MBQ * P:(mq + 1) * MBQ * P,
                                           k0:k0 + KBG * P]
                                src = src.rearrange("(g p) k -> p g k", p=P)
                                nc.sync.dma_start(st_t[:], src)
                                st_ts.append(st_t)
                            # kb/h-major, mb-minor: each psum tile's
                            # accumulation closes early so its drain overlaps
                            # the next tile's matmuls.
                            for kb in range(KBG):
                                t = agg_pool.tile([P, CB], mmdt, tag="agg",
                                                  name=f"agg{j}_{kb}")
                                for h in range(2):
                                    pst = agg_ps_pool.tile(
                                        [P, HALF], f32, tag="aggps",
                                        name=f"aggps{kb}_{h}")
                                    for mb in range(NMB):
                                        mq, ml = divmod(mb, MBQ)
                                        lhsT = st_ts[mq][:, ml,
                                                         kb * P:(kb + 1) * P]
                                        nc.tensor.matmul(
                                            pst[:],
                                            lhsT,
                                            x_tiles[mb][:, h * HALF:(h + 1) * HALF],
                                            start=(mb == 0),
                                            stop=(mb == NMB - 1),
                                        )
                                    if (kb + h) % 2 == 0:
                                        nc.vector.tensor_copy(
                                            t[:, h * HALF:(h + 1) * HALF],
                                            pst[:])
                                    else:
                                        nc.scalar.copy(
                                            t[:, h * HALF:(h + 1) * HALF],
                                            pst[:])
                                agg_sb[(j, kb)] = t

                        for i in range(BC):
                            aggT = []
                            for j in range(J):
                                tp = tp_ps_pool.tile([F, KBG * P], mmdt,
                                                     tag="tpproj",
                                                     name=f"tp{i}_{j}")
                                for kb in range(KBG):
                                    nc.tensor.transpose(
                                        tp[:, kb * P:(kb + 1) * P],
                                        agg_sb[(j, kb)][:, i * F:(i + 1) * F],
                                        ident[:])
                                at = aggT_pool.tile([F, KBG * P], mmdt,
                                                    tag="aggT",
                                                    name=f"aggT{i}_{j}")
                                if (i + j) % 2 == 0:
                                    nc.vector.tensor_copy(at[:], tp[:])
                                else:
                                    nc.scalar.copy(at[:], tp[:])
                                aggT.append(at)
                            pp = tp_ps_pool.tile([out_rows, KBG * P], f32,
                                                 tag="tpproj",
                                                 name=f"proj{i}")
                            for j in range(J):
                                nc.tensor.matmul(
                                    pp[:],
                                    w_tiles[j][:],
                                    aggT[j][:],
                                    start=(j == 0),
                                    stop=(j == J - 1),
                                )
                            nc.scalar.activation(
                                out_slice_fn(i, k0, KBG * P), pp[:], act_fn,
                                bias=bias_t[:, 0:1])

            # ---- phase 1: r|u = sigmoid(graph_conv(xs, Wr|Wu)) ----
            big_phase(
                xs_tiles, wru_t, 2 * OUT, bru_t, AF.Sigmoid,
                lambda i, k0, w: ruT_tiles[i][:, k0:k0 + w])

            # ---- boundary: xc = [input, r*state] in [m, (i,f)] layout ----
            xc_tiles = [xs_pool.tile([P, CB], mmdt, tag="xs", name=f"xc{mb}")
                        for mb in range(NMB)]
            with tc.tile_pool(name="rstp", bufs=2, space="PSUM") as rstp_pool, \
                 tc.tile_pool(name="rsT", bufs=2) as rsT_pool:
                for mb in range(NMB):
                    dst = xc_tiles[mb][:].rearrange("p (i f) -> p i f", f=F)
                    src = xin_tiles[mb][:].rearrange("p (i f) -> p i f", f=IN)
                    nc.vector.tensor_copy(dst[:, :, 0:IN], src)
                for i in range(BC):
                    stt = stT_pool.tile([OUT, N], f32, tag="stT")
                    nc.sync.dma_start(stt[:], stT_d[i])
                    rst = rsT_pool.tile([OUT, N], mmdt, tag="rsT")
                    nc.vector.tensor_mul(rst[:], ruT_tiles[i][0:OUT, :],
                                         stt[:])
                    for mb in range(NMB):
                        tp = rstp_pool.tile([P, OUT], mmdt, tag="rstp")
                        nc.tensor.transpose(tp[:], rst[:, mb * P:(mb + 1) * P],
                                            ident[0:OUT, 0:OUT])
                        if mb % 2 == 0:
                            nc.vector.tensor_copy(
                                xc_tiles[mb][:, i * F + IN:(i + 1) * F], tp[:])
                        else:
                            nc.scalar.copy(
                                xc_tiles[mb][:, i * F + IN:(i + 1) * F], tp[:])

            # ---- phase 2: c.T = tanh(proj) overwrites dead r.T rows ----
            big_phase(
                xc_tiles, wc_t, OUT, bc_t, AF.Tanh,
                lambda i, k0, w: ruT_tiles[i][0:OUT, k0:k0 + w])

            # ---- GRU combine: out = c + u*(state - c) ----
            with tc.tile_pool(name="tmp", bufs=3) as tmp_pool:
                for i in range(BC):
                    stt = stT_pool.tile([OUT, N], f32, tag="stT")
                    nc.sync.dma_start(stt[:], stT_d[i])
                    u0 = tmp_pool.tile([OUT, N], f32, tag="tmp")
                    # partition-base shift (64 -> 0) needs a DMA, not DVE
                    nc.sync.dma_start(u0[:], ruT_tiles[i][OUT:2 * OUT, :])
                    t1 = tmp_pool.tile([OUT, N], f32, tag="tmp")
                    nc.vector.tensor_sub(t1[:], stt[:], ruT_tiles[i][0:OUT, :])
                    t2 = tmp_pool.tile([OUT, N], f32, tag="tmp")
                    nc.vector.tensor_mul(t2[:], u0[:], t1[:])
                    t3 = tmp_pool.tile([OUT, N], f32, tag="tmp")
                    nc.vector.tensor_add(t3[:], ruT_tiles[i][0:OUT, :], t2[:])
                    nc.sync.dma_start(outT_d[i], t3[:])

    nc.compile()
    return nc


def _get_module():
    if "nc" not in _CACHE:
        _CACHE["nc"] = _build_module()
    return _CACHE["nc"]


def kernel(input, state, supports, Wr, br, Wu, bu, Wc, bc):
    input = np.asarray(input, np.float32)
    state = np.asarray(state, np.float32)
    supports = np.asarray(supports, np.float32)
    Wr = np.asarray(Wr, np.float32)
    br = np.asarray(br, np.float32)
    Wu = np.asarray(Wu, np.float32)
    bu = np.asarray(bu, np.float32)
    Wc = np.asarray(Wc, np.float32)
    bc = np.asarray(bc, np.float32)

    from concourse.bass_utils import run_bass_kernel_spmd

    nc = _get_module()

    mmnp = np.float16 if MM16 else np.float32
    st_host = np.ascontiguousarray(supports.transpose(0, 2, 1).astype(mmnp))
    wru = np.ascontiguousarray(np.concatenate([Wr, Wu], axis=2).astype(mmnp))
    bru = np.concatenate([br, bu]).reshape(2 * OUT, 1).astype(np.float32)
    bcc = bc.reshape(OUT, 1).astype(np.float32)
    xs_full = np.concatenate([input, state], axis=2)  # [B, N, F]

    in_maps = []
    for c in range(NCORES):
        sl = slice(c * BC, (c + 1) * BC)
        xs_c = np.ascontiguousarray(
            xs_full[sl].transpose(1, 0, 2).reshape(N, CB).astype(mmnp))
        xin_c = np.ascontiguousarray(
            input[sl].transpose(1, 0, 2).reshape(N, BC * IN))
        stT_c = np.ascontiguousarray(state[sl].transpose(0, 2, 1))
        in_maps.append({
            "st": st_host,
            "xs": xs_c,
            "xin": xin_c,
            "stT": stT_c,
            "wru": wru,
            "wc": np.ascontiguousarray(Wc.astype(mmnp)),
            "bru": bru,
            "bc": bcc,
        })

    import time
    t0 = time.monotonic()
    res = run_bass_kernel_spmd(nc, in_maps, core_ids=list(range(NCORES)))
    _CACHE["last_wall_s"] = time.monotonic() - t0

    out = np.empty((B, N, OUT), np.float32)
    for c in range(NCORES):
        outT = res.results[c]["outT"]           # [BC, OUT, N]
        out[c * BC:(c + 1) * BC] = outT.transpose(0, 2, 1)
    return out



# revision 2
# speedup vs baseline: 1.0271x; 1.0271x over previous
"""DCGRU cell Trainium2 kernel (restructured, fp16).

Math (per batch i):
  xs = [input, state]                                  [N, 66]
  r|u = sigmoid(sum_j (S[j] @ xs) @ Wru[j] + bru)      [N, 128]
  c   = tanh(sum_j S[j] @ ([input, r*state] @ Wc[j]) + bc)
  out = u*state + (1-u)*c

Key structure vs v1:
  - Phase 1 (r,u) stays aggregation-first, but the transpose/project/act
    chain for group g-1 is interleaved into group g's big matmuls so the
    in-order PE never waits on DVE/ACT copies.
  - Phase 2 (c) uses associativity: y[i] = xc[i] @ Wc (small matmuls with
    the [66, N]-layout xcT built directly from r*state — zero transposes),
    then c = sum_j S[j] @ y[i,j] accumulated in native [k, (i,o)] layout.
    bc is folded in via a rank-1 ones @ bcn matmul into the same psum.
  - u is moved to native layout with PE transposes using an identity block
    at partition base 64 (no shift DMA); combine runs per-k-block in
    native [128, (i,o)] layout, fully overlapped under the phase-2 bigs.
  - Output is written in native [N, (i,o)] fp32; host just reshapes.

Sharding: data-parallel over batch, 8 batches per core on 8 cores.
"""

import sys

if '/opt/trn_rl_repo' not in sys.path:
    sys.path.insert(0, '/opt/trn_rl_repo')

import numpy as np

B, N, IN, OUT, J = 64, 2048, 2, 64, 4
NCORES = 8
BC = B // NCORES            # 8 batches per core
F = IN + OUT                # 66
CB = BC * F                 # 528 moving columns (phase 1)
P = 128
HALF = CB // 2              # 264
NMB = N // P                # 16 m blocks
G = 8                       # k groups (256 wide)
KBG = 2                     # k blocks per group
MBQ = 8                     # m blocks per phase-1 st tile
RU = 2 * OUT                # 128
YW = J * OUT                # 256 (small-y width per batch)
NAT = BC * OUT              # 512 (native combine width)

_CACHE = {}


def _build_module():
    import concourse.tile as tile
    import concourse.mybir as mybir
    from concourse import bacc
    from concourse.masks import make_identity

    f32 = mybir.dt.float32
    f16 = mybir.dt.float16
    AF = mybir.ActivationFunctionType

    nc = bacc.Bacc("TRN2", target_bir_lowering=False, debug=False,
                   num_devices=1)

    st_d = nc.dram_tensor("st", [J, N, N], f16, kind="ExternalInput").ap()
    xs_d = nc.dram_tensor("xs", [N, CB], f16, kind="ExternalInput").ap()
    xinT_d = nc.dram_tensor("xinT", [BC, IN, N], f16, kind="ExternalInput").ap()
    stT_d = nc.dram_tensor("stT", [BC, OUT, N], f16, kind="ExternalInput").ap()
    stN_d = nc.dram_tensor("stN", [N, NAT], f16, kind="ExternalInput").ap()
    wru_d = nc.dram_tensor("wru", [J, F, RU], f16, kind="ExternalInput").ap()
    wcp_d = nc.dram_tensor("wcp", [F, YW], f16, kind="ExternalInput").ap()
    bru_d = nc.dram_tensor("bru", [RU, 1], f32, kind="ExternalInput").ap()
    bcn_d = nc.dram_tensor("bcn", [1, NAT], f16, kind="ExternalInput").ap()
    out_d = nc.dram_tensor("out", [N, NAT], f32, kind="ExternalOutput").ap()

    with tile.TileContext(nc) as tc:
        with tc.tile_pool(name="const", bufs=1) as const_pool, \
             tc.tile_pool(name="ruT", bufs=BC) as ruT_pool, \
             tc.tile_pool(name="rsT", bufs=BC) as rsT_pool:

            ident = const_pool.tile([P, P], f16, tag="ident")
            make_identity(nc, ident[:])
            # identity block at partition base 64: 1 at (64+c, c)
            ident64 = const_pool.tile([P, OUT], f16, tag="ident64")
            nc.gpsimd.memset(ident64[:], 0.0)
            nc.gpsimd.affine_select(
                out=ident64[:], in_=ident64[:],
                compare_op=mybir.AluOpType.not_equal,
                fill=1.0, base=-OUT,
                pattern=[[-1, OUT]], channel_multiplier=1)
            ones1 = const_pool.tile([1, P], f16, tag="ones1")
            nc.gpsimd.memset(ones1[:], 1.0)

            wru_t = []
            for j in range(J):
                w1 = const_pool.tile([F, RU], f16, tag=f"wru{j}")
                nc.sync.dma_start(w1[:], wru_d[j])
                wru_t.append(w1)
            wcp_t = const_pool.tile([F, YW], f16, tag="wcp")
            nc.sync.dma_start(wcp_t[:], wcp_d[:])
            bru_t = const_pool.tile([RU, 1], f32, tag="bru")
            nc.sync.dma_start(bru_t[:], bru_d[:])
            bcn_t = const_pool.tile([1, NAT], f16, tag="bcn")
            nc.sync.dma_start(bcn_t[:], bcn_d[:])

            ruT_tiles = [ruT_pool.tile([P, N], f16, tag="ruT", name=f"ruT{i}")
                         for i in range(BC)]
            # rsT rows 0:64 = (r*state).T written per group; rows 64:66 = xinT
            rsT_tiles = [rsT_pool.tile([F, N], f16, tag="rsT", name=f"rsT{i}")
                         for i in range(BC)]
            for i in range(BC):
                nc.sync.dma_start(rsT_tiles[i][OUT:F, :], xinT_d[i])

            # rotating drain engines (gpsimd cannot read PSUM)
            drain_engs = [nc.vector.tensor_copy, nc.scalar.copy]
            rot = {"n": 0}

            def drain(dst, src):
                eng = drain_engs[rot["n"] % 2]
                rot["n"] += 1
                eng(dst, src)

            # ---------------- phase 1 ----------------
            with tc.tile_pool(name="xs", bufs=NMB) as xs_pool, \
                 tc.tile_pool(name="stT", bufs=BC) as stT_pool, \
                 tc.tile_pool(name="st1", bufs=12) as st1_pool, \
                 tc.tile_pool(name="agg", bufs=16) as agg_pool, \
                 tc.tile_pool(name="aggT", bufs=6) as aggT_pool, \
                 tc.tile_pool(name="aggps", bufs=4, space="PSUM") as agg_ps, \
                 tc.tile_pool(name="tpps", bufs=2, space="PSUM") as tp_ps, \
                 tc.tile_pool(name="ppps", bufs=2, space="PSUM") as pp_ps:

                xs_tiles = []
                for mb in range(NMB):
                    t = xs_pool.tile([P, CB], f16, tag="xs")
                    nc.sync.dma_start(t[:], xs_d[mb * P:(mb + 1) * P, :])
                    xs_tiles.append(t)
                stT_tiles = []
                for i in range(BC):
                    t = stT_pool.tile([OUT, N], f16, tag="stT")
                    nc.sync.dma_start(t[:], stT_d[i])
                    stT_tiles.append(t)

                agg_sb = {}        # (g, j, kb) -> agg tile
                aggT_sb = {}       # (i,) -> [tp0_copy, tp1_copy] tiles

                def emit_big_burst(g, j, kb, st_ts):
                    ps0 = agg_ps.tile([P, HALF], f32, tag="aggps",
                                      name=f"ps{g}_{j}_{kb}_0")
                    ps1 = agg_ps.tile([P, HALF], f32, tag="aggps",
                                      name=f"ps{g}_{j}_{kb}_1")
                    for mb in range(NMB):
                        mq, ml = divmod(mb, MBQ)
                        lhsT = st_ts[j][mq][:, ml, kb * P:(kb + 1) * P]
                        nc.tensor.matmul(ps0[:], lhsT,
                                         xs_tiles[mb][:, 0:HALF],
                                         start=(mb == 0), stop=(mb == NMB - 1))
                        nc.tensor.matmul(ps1[:], lhsT,
                                         xs_tiles[mb][:, HALF:CB],
                                         start=(mb == 0), stop=(mb == NMB - 1))
                    t = agg_pool.tile([P, CB], f16, tag="agg",
                                      name=f"agg{g}_{j}_{kb}")
                    drain(t[:, 0:HALF], ps0[:])
                    drain(t[:, HALF:CB], ps1[:])
                    agg_sb[(g, j, kb)] = t

                def emit_tp(g, i):
                    # transpose agg[g] batch-i slices into aggT (f-major)
                    tps = []
                    for jj in range(2):          # j pairs
                        tp = tp_ps.tile([F, 520], f16, tag="tpps",
                                        name=f"tp{g}_{i}_{jj}")
                        n_mm = 0
                        for dj in range(2):
                            j = jj * 2 + dj
                            for kb in range(KBG):
                                c0 = (dj * 2 + kb) * P
                                nc.tensor.matmul(
                                    tp[:, c0:c0 + P],
                                    agg_sb[(g, j, kb)][:, i * F:(i + 1) * F],
                                    ident[:],
                                    start=(n_mm == 0), stop=(n_mm == 3),
                                    is_transpose=True)
                                n_mm += 1
                        tps.append(tp)
                    a0 = aggT_pool.tile([F, 512], f16, tag="aggT",
                                        name=f"aggT{i}_0")
                    a1 = aggT_pool.tile([F, 512], f16, tag="aggT",
                                        name=f"aggT{i}_1")
                    drain(a0[:], tps[0][:, 0:512])
                    drain(a1[:], tps[1][:, 0:512])
                    aggT_sb[i] = (a0, a1)

                def emit_finish(g, i):
                    # project + activate + r*state for batch i of group g
                    k0 = g * KBG * P
                    pp = pp_ps.tile([RU, 260], f32, tag="ppps",
                                    name=f"pp{g}_{i}")
                    for j in range(J):
                        jj, dj = divmod(j, 2)
                        rhs = aggT_sb[i][jj][:, dj * 256:(dj + 1) * 256]
                        nc.tensor.matmul(pp[:, 0:256], wru_t[j][:], rhs,
                                         start=(j == 0), stop=(j == J - 1))
                    nc.scalar.activation(
                        ruT_tiles[i][:, k0:k0 + 256], pp[:, 0:256],
                        AF.Sigmoid, bias=bru_t[:, 0:1])
                    nc.vector.tensor_mul(
                        rsT_tiles[i][0:OUT, k0:k0 + 256],
                        ruT_tiles[i][0:OUT, k0:k0 + 256],
                        stT_tiles[i][:, k0:k0 + 256])

                def chain_step(g, s):
                    # s in [0..8]: step s of the pipelined chain for group g
                    if g < 0:
                        return
                    if s < BC:
                        emit_tp(g, s)
                    if s >= 1:
                        emit_finish(g, s - 1)

                def dma_st_group(g):
                    st_ts = []
                    k0 = g * KBG * P
                    for j in range(J):
                        per_j = []
                        for mq in range(NMB // MBQ):
                            st_t = st1_pool.tile([P, MBQ, KBG * P], f16,
                                                 tag="st1")
                            src = st_d[j, mq * MBQ * P:(mq + 1) * MBQ * P,
                                       k0:k0 + KBG * P]
                            src = src.rearrange("(g p) k -> p g k", p=P)
                            nc.sync.dma_start(st_t[:], src)
                            per_j.append(st_t)
                        st_ts.append(per_j)
                    return st_ts

                st_next = dma_st_group(0)
                for g in range(G):
                    st_cur = st_next
                    if g + 1 < G:
                        st_next = dma_st_group(g + 1)
                    b = 0
                    for j in range(J):
                        for kb in range(KBG):
                            emit_big_burst(g, j, kb, st_cur)
                            chain_step(g - 1, b)
                            b += 1
                    chain_step(g - 1, 8)
                # tail: chain for last group
                for s in range(BC + 1):
                    chain_step(G - 1, s)

            # ---------------- phase 2 ----------------
            with tc.tile_pool(name="y", bufs=NMB) as y_pool, \
                 tc.tile_pool(name="st2", bufs=8) as st2_pool, \
                 tc.tile_pool(name="unat", bufs=3) as unat_pool, \
                 tc.tile_pool(name="stN", bufs=NMB) as stN_pool, \
                 tc.tile_pool(name="csb", bufs=3) as csb_pool, \
                 tc.tile_pool(name="tmp", bufs=4) as tmp_pool, \
                 tc.tile_pool(name="ost", bufs=3) as ost_pool, \
                 tc.tile_pool(name="utpps", bufs=2, space="PSUM") as utp_ps:

                stN_tiles = []
                for kb in range(NMB):
                    t = stN_pool.tile([P, NAT], f16, tag="stN")
                    nc.sync.dma_start(t[:], stN_d[kb * P:(kb + 1) * P, :])
                    stN_tiles.append(t)

                y_tiles = [y_pool.tile([P, J * NAT], f16, tag="y",
                                       name=f"y{mb}")
                           for mb in range(NMB)]

                def yv_drain(mb, i):
                    v = y_tiles[mb][:].rearrange("p (j i o) -> p j i o",
                                                 j=J, i=BC)
                    return v[:, :, i, :]

                def yv_big(mb, j):
                    v = y_tiles[mb][:].rearrange("p (j io) -> p j io", j=J)
                    return v[:, j, :]

                u_nat = [unat_pool.tile([P, NAT], f16, tag="unat",
                                        name=f"unat{x}") for x in range(3)]

                def emit_utp(i, kb):
                    up = utp_ps.tile([P, OUT], f16, tag="utpps",
                                     name=f"utp{kb}_{i}")
                    nc.tensor.matmul(
                        up[:], ruT_tiles[i][OUT:RU, kb * P:(kb + 1) * P],
                        ident64[OUT:P, :], is_transpose=True)
                    drain(u_nat[kb % 3][:, i * OUT:(i + 1) * OUT], up[:])

                # small-y window (+ u transposes for kb 0 and 1)
                with tc.tile_pool(name="yps", bufs=4, space="PSUM") as y_ps:
                    utp_pre = [(i, kb) for kb in range(2) for i in range(BC)]
                    un = 0
                    for i in range(BC):
                        for mb in range(NMB):
                            yp = y_ps.tile([P, 260], f32, tag="yps",
                                           name=f"yps{i}_{mb}")
                            nc.tensor.matmul(
                                yp[:, 0:YW],
                                rsT_tiles[i][:, mb * P:(mb + 1) * P],
                                wcp_t[:])
                            drain(yv_drain(mb, i),
                                  yp[:, 0:YW].rearrange("p (j o) -> p j o",
                                                        o=OUT))
                            if mb % 8 == 7 and un < len(utp_pre):
                                emit_utp(*utp_pre[un])
                                un += 1

                def dma_st2(kb):
                    ts = []
                    for j in range(J):
                        t = st2_pool.tile([P, NMB, P], f16, tag="st2")
                        src = st_d[j, :, kb * P:(kb + 1) * P]
                        src = src.rearrange("(q p) k -> p q k", p=P)
                        nc.sync.dma_start(t[:], src)
                        ts.append(t)
                    return ts

                with tc.tile_pool(name="cps", bufs=3, space="PSUM") as c_ps:
                    _phase2_bigs(tc, nc, dma_st2, c_ps, csb_pool, tmp_pool,
                                 ost_pool, stN_tiles, u_nat, emit_utp,
                                 yv_big, ones1, bcn_t, out_d, AF, f16, f32)

    nc.compile()
    return nc


def _phase2_bigs(tc, nc, dma_st2, c_ps, csb_pool, tmp_pool, ost_pool,
                 stN_tiles, u_nat, emit_utp, yv_big, ones1, bcn_t, out_d,
                 AF, f16, f32):
                st2_next = dma_st2(0)
                st2_next2 = dma_st2(1)
                for kb in range(NMB):
                    st2_cur = st2_next
                    st2_next = st2_next2
                    st2_next2 = dma_st2(kb + 2) if kb + 2 < NMB else None

                    cp = c_ps.tile([P, NAT], f32, tag="cps", name=f"cps{kb}")
                    for j in range(J):
                        for mb in range(NMB):
                            nc.tensor.matmul(
                                cp[:], st2_cur[j][:, mb, :], yv_big(mb, j),
                                start=(j == 0 and mb == 0), stop=False)
                        # sprinkle u-transposes for kb+2
                        if kb + 2 < NMB and j < 4:
                            for di in range(2):
                                emit_utp(j * 2 + di, kb + 2)
                    nc.tensor.matmul(cp[:], ones1[:], bcn_t[:],
                                     start=False, stop=True)

                    c_sb = csb_pool.tile([P, NAT], f16, tag="csb")
                    nc.scalar.activation(c_sb[:], cp[:], AF.Tanh)
                    t1 = tmp_pool.tile([P, NAT], f16, tag="tmp")
                    nc.vector.tensor_sub(t1[:], stN_tiles[kb][:], c_sb[:])
                    t2 = tmp_pool.tile([P, NAT], f16, tag="tmp")
                    nc.vector.tensor_mul(t2[:], u_nat[kb % 3][:], t1[:])
                    t3 = ost_pool.tile([P, NAT], f32, tag="ost")
                    nc.vector.tensor_add(t3[:], c_sb[:], t2[:])
                    nc.sync.dma_start(out_d[kb * P:(kb + 1) * P, :], t3[:])


def _get_module():
    if "nc" not in _CACHE:
        _CACHE["nc"] = _build_module()
    return _CACHE["nc"]


def kernel(input, state, supports, Wr, br, Wu, bu, Wc, bc):
    input = np.asarray(input, np.float32)
    state = np.asarray(state, np.float32)
    supports = np.asarray(supports, np.float32)
    Wr = np.asarray(Wr, np.float32)
    br = np.asarray(br, np.float32)
    Wu = np.asarray(Wu, np.float32)
    bu = np.asarray(bu, np.float32)
    Wc = np.asarray(Wc, np.float32)
    bc = np.asarray(bc, np.float32)

    from concourse.bass_utils import run_bass_kernel_spmd

    nc = _get_module()

    st_host = np.ascontiguousarray(supports.transpose(0, 2, 1).astype(np.float16))
    wru = np.ascontiguousarray(
        np.concatenate([Wr, Wu], axis=2).astype(np.float16))
    # Wc with feature rows permuted [state(2:66); input(0:2)], flattened (j,o)
    wcp = np.ascontiguousarray(
        np.concatenate([Wc[:, IN:F, :], Wc[:, 0:IN, :]], axis=1)
        .transpose(1, 0, 2).reshape(F, YW).astype(np.float16))
    bru = np.concatenate([br, bu]).reshape(RU, 1).astype(np.float32)
    bcn = np.tile(bc, BC).reshape(1, NAT).astype(np.float16)
    xs_full = np.concatenate([input, state], axis=2)  # [B, N, F]

    in_maps = []
    for c in range(NCORES):
        sl = slice(c * BC, (c + 1) * BC)
        xs_c = np.ascontiguousarray(
            xs_full[sl].transpose(1, 0, 2).reshape(N, CB).astype(np.float16))
        xinT_c = np.ascontiguousarray(
            input[sl].transpose(0, 2, 1).astype(np.float16))
        stT_c = np.ascontiguousarray(
            state[sl].transpose(0, 2, 1).astype(np.float16))
        stN_c = np.ascontiguousarray(
            state[sl].transpose(1, 0, 2).reshape(N, NAT).astype(np.float16))
        in_maps.append({
            "st": st_host,
            "xs": xs_c,
            "xinT": xinT_c,
            "stT": stT_c,
            "stN": stN_c,
            "wru": wru,
            "wcp": wcp,
            "bru": bru,
            "bcn": bcn,
        })

    import time
    t0 = time.monotonic()
    res = run_bass_kernel_spmd(nc, in_maps, core_ids=list(range(NCORES)))
    _CACHE["last_wall_s"] = time.monotonic() - t0

    out = np.empty((B, N, OUT), np.float32)
    for c in range(NCORES):
        o = res.results[c]["out"]               # [N, NAT]
        out[c * BC:(c + 1) * BC] = o.reshape(N, BC, OUT).transpose(1, 0, 2)
    return out


# revision 3
# speedup vs baseline: 1.0277x; 1.0006x over previous
"""DCGRU cell Trainium2 kernel, v2 (restructured, fp16).

Math (per batch i):
  xs = [input, state]                                  [N, 66]
  r|u = sigmoid(sum_j (S[j] @ xs) @ Wru[j] + bru)      [N, 128]
  c   = tanh(sum_j S[j] @ ([input, r*state] @ Wc[j]) + bc)
  out = u*state + (1-u)*c

Key structure vs v1:
  - Phase 1 (r,u) stays aggregation-first, but the transpose/project/act
    chain for group g-1 is interleaved into group g's big matmuls so the
    in-order PE never waits on DVE/ACT copies.
  - Phase 2 (c) uses associativity: y[i] = xc[i] @ Wc (small matmuls with
    the [66, N]-layout xcT built directly from r*state — zero transposes),
    then c = sum_j S[j] @ y[i,j] accumulated in native [k, (i,o)] layout.
    bc is folded in via a rank-1 ones @ bcn matmul into the same psum.
  - u is moved to native layout with PE transposes using an identity block
    at partition base 64 (no shift DMA); combine runs per-k-block in
    native [128, (i,o)] layout, fully overlapped under the phase-2 bigs.
  - Output is written in native [N, (i,o)] fp32; host just reshapes.

Sharding: data-parallel over batch, 8 batches per core on 8 cores.
"""

import sys

if '/opt/trn_rl_repo' not in sys.path:
    sys.path.insert(0, '/opt/trn_rl_repo')

import numpy as np

B, N, IN, OUT, J = 64, 2048, 2, 64, 4
NCORES = 8
BC = B // NCORES            # 8 batches per core
F = IN + OUT                # 66
CB = BC * F                 # 528 moving columns (phase 1)
P = 128
HALF = CB // 2              # 264
NMB = N // P                # 16 m blocks
G = 8                       # k groups (256 wide)
KBG = 2                     # k blocks per group
MBQ = 8                     # m blocks per phase-1 st tile
RU = 2 * OUT                # 128
YW = J * OUT                # 256 (small-y width per batch)
NAT = BC * OUT              # 512 (native combine width)

_CACHE = {}


def _build_module():
    import concourse.tile as tile
    import concourse.mybir as mybir
    from concourse import bacc
    from concourse.masks import make_identity

    f32 = mybir.dt.float32
    f16 = mybir.dt.float16
    AF = mybir.ActivationFunctionType

    nc = bacc.Bacc("TRN2", target_bir_lowering=False, debug=False,
                   num_devices=1)

    st_d = nc.dram_tensor("st", [J, N, N], f16, kind="ExternalInput").ap()
    xs_d = nc.dram_tensor("xs", [N, CB], f16, kind="ExternalInput").ap()
    xinT_d = nc.dram_tensor("xinT", [BC, IN, N], f16, kind="ExternalInput").ap()
    stT_d = nc.dram_tensor("stT", [BC, OUT, N], f16, kind="ExternalInput").ap()
    stN_d = nc.dram_tensor("stN", [N, NAT], f16, kind="ExternalInput").ap()
    wru_d = nc.dram_tensor("wru", [J, F, RU], f16, kind="ExternalInput").ap()
    wcp_d = nc.dram_tensor("wcp", [F, YW], f16, kind="ExternalInput").ap()
    bru_d = nc.dram_tensor("bru", [RU, 1], f32, kind="ExternalInput").ap()
    bcn_d = nc.dram_tensor("bcn", [1, NAT], f16, kind="ExternalInput").ap()
    out_d = nc.dram_tensor("out", [N, NAT], f32, kind="ExternalOutput").ap()

    with tile.TileContext(nc) as tc:
        with tc.tile_pool(name="const", bufs=1) as const_pool, \
             tc.tile_pool(name="ruT", bufs=BC) as ruT_pool, \
             tc.tile_pool(name="rsT", bufs=BC) as rsT_pool:

            ident = const_pool.tile([P, P], f16, tag="ident")
            make_identity(nc, ident[:])
            # identity block at partition base 64: 1 at (64+c, c)
            ident64 = const_pool.tile([P, OUT], f16, tag="ident64")
            nc.gpsimd.memset(ident64[:], 0.0)
            nc.gpsimd.affine_select(
                out=ident64[:], in_=ident64[:],
                compare_op=mybir.AluOpType.not_equal,
                fill=1.0, base=-OUT,
                pattern=[[-1, OUT]], channel_multiplier=1)
            ones1 = const_pool.tile([1, P], f16, tag="ones1")
            nc.gpsimd.memset(ones1[:], 1.0)

            wru_t = []
            for j in range(J):
                w1 = const_pool.tile([F, RU], f16, tag=f"wru{j}")
                nc.sync.dma_start(w1[:], wru_d[j])
                wru_t.append(w1)
            wcp_t = const_pool.tile([F, YW], f16, tag="wcp")
            nc.sync.dma_start(wcp_t[:], wcp_d[:])
            bru_t = const_pool.tile([RU, 1], f32, tag="bru")
            nc.sync.dma_start(bru_t[:], bru_d[:])
            bcn_t = const_pool.tile([1, NAT], f16, tag="bcn")
            nc.sync.dma_start(bcn_t[:], bcn_d[:])

            ruT_tiles = [ruT_pool.tile([P, N], f16, tag="ruT", name=f"ruT{i}")
                         for i in range(BC)]
            # rsT rows 0:64 = (r*state).T written per group; rows 64:66 = xinT
            rsT_tiles = [rsT_pool.tile([F, N], f16, tag="rsT", name=f"rsT{i}")
                         for i in range(BC)]

            # rotating drain engines (gpsimd cannot read PSUM)
            drain_engs = [nc.vector.tensor_copy, nc.scalar.copy]
            rot = {"n": 0}

            def drain(dst, src):
                eng = drain_engs[rot["n"] % 2]
                rot["n"] += 1
                eng(dst, src)

            # ---------------- phase 1 ----------------
            with tc.tile_pool(name="xs", bufs=NMB) as xs_pool, \
                 tc.tile_pool(name="stT", bufs=BC) as stT_pool, \
                 tc.tile_pool(name="st1", bufs=12) as st1_pool, \
                 tc.tile_pool(name="agg", bufs=16) as agg_pool, \
                 tc.tile_pool(name="aggT", bufs=6) as aggT_pool, \
                 tc.tile_pool(name="aggps", bufs=4, space="PSUM") as agg_ps, \
                 tc.tile_pool(name="tpps", bufs=2, space="PSUM") as tp_ps, \
                 tc.tile_pool(name="ppps", bufs=2, space="PSUM") as pp_ps:

                def dma_st_group(g):
                    st_ts = []
                    k0 = g * KBG * P
                    for j in range(J):
                        per_j = []
                        for mq in range(NMB // MBQ):
                            st_t = st1_pool.tile([P, MBQ, KBG * P], f16,
                                                 tag="st1")
                            src = st_d[j, mq * MBQ * P:(mq + 1) * MBQ * P,
                                       k0:k0 + KBG * P]
                            src = src.rearrange("(g p) k -> p g k", p=P)
                            nc.gpsimd.dma_start(st_t[:], src)
                            per_j.append(st_t)
                        st_ts.append(per_j)
                    return st_ts

                # critical-path DMAs first: the first big burst contracts
                # over every xs tile, so xs must land before st group 0.
                xs_tiles = []
                for mb in range(NMB):
                    t = xs_pool.tile([P, CB], f16, tag="xs")
                    nc.sync.dma_start(t[:], xs_d[mb * P:(mb + 1) * P, :])
                    xs_tiles.append(t)
                st_next = dma_st_group(0)
                # stT/xinT feed only the DVE rst chain (slack until the
                # junction) — defer their DMA issue to group 2.
                stT_tiles = [stT_pool.tile([OUT, N], f16, tag="stT",
                                           name=f"stT{i}")
                             for i in range(BC)]

                def dma_stT():
                    for i in range(BC):
                        nc.gpsimd.dma_start(stT_tiles[i][:], stT_d[i])
                        nc.gpsimd.dma_start(rsT_tiles[i][OUT:F, :], xinT_d[i])

                agg_sb = {}        # (g, j, kb) -> agg tile
                aggT_sb = {}       # (i,) -> [tp0_copy, tp1_copy] tiles

                def emit_big_burst(g, j, kb, st_ts):
                    ps0 = agg_ps.tile([P, HALF], f32, tag="aggps",
                                      name=f"ps{g}_{j}_{kb}_0")
                    ps1 = agg_ps.tile([P, HALF], f32, tag="aggps",
                                      name=f"ps{g}_{j}_{kb}_1")
                    for mb in range(NMB):
                        mq, ml = divmod(mb, MBQ)
                        lhsT = st_ts[j][mq][:, ml, kb * P:(kb + 1) * P]
                        nc.tensor.matmul(ps0[:], lhsT,
                                         xs_tiles[mb][:, 0:HALF],
                                         start=(mb == 0), stop=(mb == NMB - 1))
                        nc.tensor.matmul(ps1[:], lhsT,
                                         xs_tiles[mb][:, HALF:CB],
                                         start=(mb == 0), stop=(mb == NMB - 1))
                    t = agg_pool.tile([P, CB], f16, tag="agg",
                                      name=f"agg{g}_{j}_{kb}")
                    drain(t[:, 0:HALF], ps0[:])
                    drain(t[:, HALF:CB], ps1[:])
                    agg_sb[(g, j, kb)] = t

                def emit_tp(g, i):
                    # transpose agg[g] batch-i slices into aggT (f-major)
                    tps = []
                    for jj in range(2):          # j pairs
                        tp = tp_ps.tile([F, 520], f16, tag="tpps",
                                        name=f"tp{g}_{i}_{jj}")
                        n_mm = 0
                        for dj in range(2):
                            j = jj * 2 + dj
                            for kb in range(KBG):
                                c0 = (dj * 2 + kb) * P
                                nc.tensor.matmul(
                                    tp[:, c0:c0 + P],
                                    agg_sb[(g, j, kb)][:, i * F:(i + 1) * F],
                                    ident[:],
                                    start=(n_mm == 0), stop=(n_mm == 3),
                                    is_transpose=True)
                                n_mm += 1
                        tps.append(tp)
                    a0 = aggT_pool.tile([F, 512], f16, tag="aggT",
                                        name=f"aggT{i}_0")
                    a1 = aggT_pool.tile([F, 512], f16, tag="aggT",
                                        name=f"aggT{i}_1")
                    drain(a0[:], tps[0][:, 0:512])
                    drain(a1[:], tps[1][:, 0:512])
                    aggT_sb[i] = (a0, a1)

                def emit_finish(g, i):
                    # project + activate + r*state for batch i of group g
                    k0 = g * KBG * P
                    pp = pp_ps.tile([RU, 260], f32, tag="ppps",
                                    name=f"pp{g}_{i}")
                    for j in range(J):
                        jj, dj = divmod(j, 2)
                        rhs = aggT_sb[i][jj][:, dj * 256:(dj + 1) * 256]
                        nc.tensor.matmul(pp[:, 0:256], wru_t[j][:], rhs,
                                         start=(j == 0), stop=(j == J - 1))
                    nc.scalar.activation(
                        ruT_tiles[i][:, k0:k0 + 256], pp[:, 0:256],
                        AF.Sigmoid, bias=bru_t[:, 0:1])
                    nc.vector.tensor_mul(
                        rsT_tiles[i][0:OUT, k0:k0 + 256],
                        ruT_tiles[i][0:OUT, k0:k0 + 256],
                        stT_tiles[i][:, k0:k0 + 256])

                def chain_step(g, s):
                    # s in [0..8]: step s of the pipelined chain for group g
                    if g < 0:
                        return
                    if s < BC:
                        emit_tp(g, s)
                    if s >= 1:
                        emit_finish(g, s - 1)

                for g in range(G):
                    st_cur = st_next
                    if g + 1 < G:
                        st_next = dma_st_group(g + 1)
                    if g == 0:
                        dma_stT()
                    b = 0
                    for j in range(J):
                        for kb in range(KBG):
                            emit_big_burst(g, j, kb, st_cur)
                            chain_step(g - 1, b)
                            b += 1
                    chain_step(g - 1, 8)
                # tail: chain for last group
                for s in range(BC + 1):
                    chain_step(G - 1, s)

            # ---------------- phase 2 ----------------
            with tc.tile_pool(name="y", bufs=NMB) as y_pool, \
                 tc.tile_pool(name="st2", bufs=8) as st2_pool, \
                 tc.tile_pool(name="unat", bufs=3) as unat_pool, \
                 tc.tile_pool(name="stN", bufs=NMB) as stN_pool, \
                 tc.tile_pool(name="csb", bufs=3) as csb_pool, \
                 tc.tile_pool(name="tmp", bufs=4) as tmp_pool, \
                 tc.tile_pool(name="ost", bufs=3) as ost_pool, \
                 tc.tile_pool(name="utpps", bufs=2, space="PSUM") as utp_ps:

                stN_tiles = []
                for kb in range(NMB):
                    t = stN_pool.tile([P, NAT], f16, tag="stN")
                    nc.gpsimd.dma_start(t[:], stN_d[kb * P:(kb + 1) * P, :])
                    stN_tiles.append(t)

                y_tiles = [y_pool.tile([P, J * NAT], f16, tag="y",
                                       name=f"y{mb}")
                           for mb in range(NMB)]

                def yv_drain(mb, i):
                    v = y_tiles[mb][:].rearrange("p (j i o) -> p j i o",
                                                 j=J, i=BC)
                    return v[:, :, i, :]

                def yv_big(mb, j):
                    v = y_tiles[mb][:].rearrange("p (j io) -> p j io", j=J)
                    return v[:, j, :]

                u_nat = [unat_pool.tile([P, NAT], f16, tag="unat",
                                        name=f"unat{x}") for x in range(3)]

                def emit_utp(i, kb):
                    up = utp_ps.tile([P, OUT], f16, tag="utpps",
                                     name=f"utp{kb}_{i}")
                    nc.tensor.matmul(
                        up[:], ruT_tiles[i][OUT:RU, kb * P:(kb + 1) * P],
                        ident64[OUT:P, :], is_transpose=True)
                    drain(u_nat[kb % 3][:, i * OUT:(i + 1) * OUT], up[:])

                # small-y window (+ u transposes for kb 0 and 1)
                with tc.tile_pool(name="yps", bufs=4, space="PSUM") as y_ps:
                    utp_pre = [(i, kb) for kb in range(2) for i in range(BC)]
                    un = 0
                    for i in range(BC):
                        for mb in range(NMB):
                            yp = y_ps.tile([P, 260], f32, tag="yps",
                                           name=f"yps{i}_{mb}")
                            nc.tensor.matmul(
                                yp[:, 0:YW],
                                rsT_tiles[i][:, mb * P:(mb + 1) * P],
                                wcp_t[:])
                            drain(yv_drain(mb, i),
                                  yp[:, 0:YW].rearrange("p (j o) -> p j o",
                                                        o=OUT))
                            if mb % 8 == 7 and un < len(utp_pre):
                                emit_utp(*utp_pre[un])
                                un += 1

                def dma_st2(kb):
                    ts = []
                    for j in range(J):
                        t = st2_pool.tile([P, NMB, P], f16, tag="st2")
                        src = st_d[j, :, kb * P:(kb + 1) * P]
                        src = src.rearrange("(q p) k -> p q k", p=P)
                        nc.gpsimd.dma_start(t[:], src)
                        ts.append(t)
                    return ts

                with tc.tile_pool(name="cps", bufs=3, space="PSUM") as c_ps:
                    _phase2_bigs(tc, nc, dma_st2, c_ps, csb_pool, tmp_pool,
                                 ost_pool, stN_tiles, u_nat, emit_utp,
                                 yv_big, ones1, bcn_t, out_d, AF, f16, f32)

    nc.compile()
    return nc


def _phase2_bigs(tc, nc, dma_st2, c_ps, csb_pool, tmp_pool, ost_pool,
                 stN_tiles, u_nat, emit_utp, yv_big, ones1, bcn_t, out_d,
                 AF, f16, f32):
                st2_next = dma_st2(0)
                st2_next2 = dma_st2(1)
                for kb in range(NMB):
                    st2_cur = st2_next
                    st2_next = st2_next2
                    st2_next2 = dma_st2(kb + 2) if kb + 2 < NMB else None

                    cp = c_ps.tile([P, NAT], f32, tag="cps", name=f"cps{kb}")
                    for j in range(J):
                        for mb in range(NMB):
                            nc.tensor.matmul(
                                cp[:], st2_cur[j][:, mb, :], yv_big(mb, j),
                                start=(j == 0 and mb == 0), stop=False)
                        # sprinkle u-transposes for kb+2
                        if kb + 2 < NMB and j < 4:
                            for di in range(2):
                                emit_utp(j * 2 + di, kb + 2)
                    nc.tensor.matmul(cp[:], ones1[:], bcn_t[:],
                                     start=False, stop=True)

                    # last block: split the drain chain in half so the
                    # act/combine/DMA tail after the final matmul shrinks
                    halves = ((0, NAT),) if kb < NMB - 1 else \
                        ((0, NAT // 2), (NAT // 2, NAT))
                    c_sb = csb_pool.tile([P, NAT], f16, tag="csb")
                    t3 = ost_pool.tile([P, NAT], f32, tag="ost")
                    for (c0, c1) in halves:
                        nc.scalar.activation(c_sb[:, c0:c1], cp[:, c0:c1],
                                             AF.Tanh)
                        w = c1 - c0
                        t1 = tmp_pool.tile([P, NAT], f16, tag="tmp")
                        nc.vector.tensor_sub(t1[:, 0:w],
                                             stN_tiles[kb][:, c0:c1],
                                             c_sb[:, c0:c1])
                        t2 = tmp_pool.tile([P, NAT], f16, tag="tmp")
                        nc.vector.tensor_mul(t2[:, 0:w],
                                             u_nat[kb % 3][:, c0:c1],
                                             t1[:, 0:w])
                        nc.vector.tensor_add(t3[:, c0:c1], c_sb[:, c0:c1],
                                             t2[:, 0:w])
                        nc.sync.dma_start(out_d[kb * P:(kb + 1) * P, c0:c1],
                                          t3[:, c0:c1])


def _get_module():
    if "nc" not in _CACHE:
        _CACHE["nc"] = _build_module()
    return _CACHE["nc"]


def kernel(input, state, supports, Wr, br, Wu, bu, Wc, bc):
    input = np.asarray(input, np.float32)
    state = np.asarray(state, np.float32)
    supports = np.asarray(supports, np.float32)
    Wr = np.asarray(Wr, np.float32)
    br = np.asarray(br, np.float32)
    Wu = np.asarray(Wu, np.float32)
    bu = np.asarray(bu, np.float32)
    Wc = np.asarray(Wc, np.float32)
    bc = np.asarray(bc, np.float32)

    from concourse.bass_utils import run_bass_kernel_spmd

    nc = _get_module()

    st_host = np.ascontiguousarray(supports.transpose(0, 2, 1).astype(np.float16))
    wru = np.ascontiguousarray(
        np.concatenate([Wr, Wu], axis=2).astype(np.float16))
    # Wc with feature rows permuted [state(2:66); input(0:2)], flattened (j,o)
    wcp = np.ascontiguousarray(
        np.concatenate([Wc[:, IN:F, :], Wc[:, 0:IN, :]], axis=1)
        .transpose(1, 0, 2).reshape(F, YW).astype(np.float16))
    bru = np.concatenate([br, bu]).reshape(RU, 1).astype(np.float32)
    bcn = np.tile(bc, BC).reshape(1, NAT).astype(np.float16)
    xs_full = np.concatenate([input, state], axis=2)  # [B, N, F]

    in_maps = []
    for c in range(NCORES):
        sl = slice(c * BC, (c + 1) * BC)
        xs_c = np.ascontiguousarray(
            xs_full[sl].transpose(1, 0, 2).reshape(N, CB).astype(np.float16))
        xinT_c = np.ascontiguousarray(
            input[sl].transpose(0, 2, 1).astype(np.float16))
        stT_c = np.ascontiguousarray(
            state[sl].transpose(0, 2, 1).astype(np.float16))
        stN_c = np.ascontiguousarray(
            state[sl].transpose(1, 0, 2).reshape(N, NAT).astype(np.float16))
        in_maps.append({
            "st": st_host,
            "xs": xs_c,
            "xinT": xinT_c,
            "stT": stT_c,
            "stN": stN_c,
            "wru": wru,
            "wcp": wcp,
            "bru": bru,
            "bcn": bcn,
        })

    import time
    t0 = time.monotonic()
    res = run_bass_kernel_spmd(nc, in_maps, core_ids=list(range(NCORES)))
    _CACHE["last_wall_s"] = time.monotonic() - t0

    out = np.empty((B, N, OUT), np.float32)
    for c in range(NCORES):
        o = res.results[c]["out"]               # [N, NAT]
        out[c * BC:(c + 1) * BC] = o.reshape(N, BC, OUT).transpose(1, 0, 2)
    return out


# revision 5
# speedup vs baseline: 1.0448x; 1.0167x over previous
"""DCGRU cell Trainium2 kernel, v2 (restructured, fp16).

Math (per batch i):
  xs = [input, state]                                  [N, 66]
  r|u = sigmoid(sum_j (S[j] @ xs) @ Wru[j] + bru)      [N, 128]
  c   = tanh(sum_j S[j] @ ([input, r*state] @ Wc[j]) + bc)
  out = u*state + (1-u)*c

Key structure vs v1:
  - Phase 1 (r,u) stays aggregation-first, but the transpose/project/act
    chain for group g-1 is interleaved into group g's big matmuls so the
    in-order PE never waits on DVE/ACT copies.
  - Phase 2 (c) uses associativity: y[i] = xc[i] @ Wc (small matmuls with
    the [66, N]-layout xcT built directly from r*state — zero transposes),
    then c = sum_j S[j] @ y[i,j] accumulated in native [k, (i,o)] layout.
    bc is folded in via a rank-1 ones @ bcn matmul into the same psum.
  - u is moved to native layout with PE transposes using an identity block
    at partition base 64 (no shift DMA); combine runs per-k-block in
    native [128, (i,o)] layout, fully overlapped under the phase-2 bigs.
  - Output is written in native [N, (i,o)] fp32; host just reshapes.

Sharding: data-parallel over batch, 8 batches per core on 8 cores.
"""

import sys

if '/opt/trn_rl_repo' not in sys.path:
    sys.path.insert(0, '/opt/trn_rl_repo')

import numpy as np

B, N, IN, OUT, J = 64, 2048, 2, 64, 4
NCORES = 8
BC = B // NCORES            # 8 batches per core
F = IN + OUT                # 66
CB = BC * F                 # 528 moving columns (phase 1)
P = 128
HALF = CB // 2              # 264
NMB = N // P                # 16 m blocks
G = 8                       # k groups (256 wide)
KBG = 2                     # k blocks per group
MBQ = 8                     # m blocks per phase-1 st tile
RU = 2 * OUT                # 128
YW = J * OUT                # 256 (small-y width per batch)
NAT = BC * OUT              # 512 (native combine width)

_CACHE = {}


def _build_module(bc_zero=False):
    import concourse.tile as tile
    import concourse.mybir as mybir
    from concourse import bacc
    from concourse.masks import make_identity

    f32 = mybir.dt.float32
    f16 = mybir.dt.float16
    AF = mybir.ActivationFunctionType

    nc = bacc.Bacc("TRN2", target_bir_lowering=False, debug=False,
                   num_devices=1)

    st_d = nc.dram_tensor("st", [J, N, N], f16, kind="ExternalInput").ap()
    xs_d = nc.dram_tensor("xs", [N, CB], f16, kind="ExternalInput").ap()
    xinT_d = nc.dram_tensor("xinT", [BC, IN, N], f16, kind="ExternalInput").ap()
    stT_d = nc.dram_tensor("stT", [BC, OUT, N], f16, kind="ExternalInput").ap()
    stN_d = nc.dram_tensor("stN", [N, NAT], f16, kind="ExternalInput").ap()
    wru_d = nc.dram_tensor("wru", [J, F, RU], f16, kind="ExternalInput").ap()
    wcp_d = nc.dram_tensor("wcp", [F, YW], f16, kind="ExternalInput").ap()
    bru_d = nc.dram_tensor("bru", [RU, 1], f32, kind="ExternalInput").ap()
    bcn_d = nc.dram_tensor("bcn", [1, NAT], f16, kind="ExternalInput").ap()
    out_d = nc.dram_tensor("out", [N, NAT], f16, kind="ExternalOutput").ap()

    with tile.TileContext(nc) as tc:
        with tc.tile_pool(name="const", bufs=1) as const_pool, \
             tc.tile_pool(name="ruT", bufs=BC) as ruT_pool, \
             tc.tile_pool(name="rsT", bufs=BC) as rsT_pool:

            ident = const_pool.tile([P, P], f16, tag="ident")
            make_identity(nc, ident[:])
            # identity block at partition base 64: 1 at (64+c, c)
            ident64 = const_pool.tile([P, OUT], f16, tag="ident64")
            nc.gpsimd.memset(ident64[:], 0.0)
            nc.gpsimd.affine_select(
                out=ident64[:], in_=ident64[:],
                compare_op=mybir.AluOpType.not_equal,
                fill=1.0, base=-OUT,
                pattern=[[-1, OUT]], channel_multiplier=1)
            ones1 = const_pool.tile([1, P], f16, tag="ones1")
            nc.gpsimd.memset(ones1[:], 1.0)
            ones_nat = const_pool.tile([P, NAT], f16, tag="ones_nat")
            nc.gpsimd.memset(ones_nat[:], 1.0)

            wru_t = []
            for j in range(J):
                w1 = const_pool.tile([F, RU], f16, tag=f"wru{j}")
                nc.sync.dma_start(w1[:], wru_d[j])
                wru_t.append(w1)
            wcp_t = const_pool.tile([F, YW], f16, tag="wcp")
            nc.sync.dma_start(wcp_t[:], wcp_d[:])
            bru_t = const_pool.tile([RU, 1], f32, tag="bru")
            nc.sync.dma_start(bru_t[:], bru_d[:])
            bcn_t = const_pool.tile([1, NAT], f16, tag="bcn")
            nc.sync.dma_start(bcn_t[:], bcn_d[:])

            ruT_tiles = [ruT_pool.tile([P, N], f16, tag="ruT", name=f"ruT{i}")
                         for i in range(BC)]
            # rsT rows 0:64 = (r*state).T written per group; rows 64:66 = xinT
            rsT_tiles = [rsT_pool.tile([F, N], f16, tag="rsT", name=f"rsT{i}")
                         for i in range(BC)]

            # rotating drain engines (gpsimd cannot read PSUM)
            drain_engs = [nc.vector.tensor_copy, nc.scalar.copy]
            rot = {"n": 0}

            def drain(dst, src):
                eng = drain_engs[rot["n"] % 2]
                rot["n"] += 1
                eng(dst, src)

            # ---------------- phase 1 ----------------
            with tc.tile_pool(name="xs", bufs=NMB) as xs_pool, \
                 tc.tile_pool(name="stT", bufs=BC) as stT_pool, \
                 tc.tile_pool(name="st1", bufs=12) as st1_pool, \
                 tc.tile_pool(name="agg", bufs=16) as agg_pool, \
                 tc.tile_pool(name="aggT", bufs=6) as aggT_pool, \
                 tc.tile_pool(name="aggps", bufs=4, space="PSUM") as agg_ps, \
                 tc.tile_pool(name="tpps", bufs=2, space="PSUM") as tp_ps, \
                 tc.tile_pool(name="ppps", bufs=2, space="PSUM") as pp_ps:

                def dma_st_group(g):
                    st_ts = []
                    k0 = g * KBG * P
                    for j in range(J):
                        per_j = []
                        for mq in range(NMB // MBQ):
                            st_t = st1_pool.tile([P, MBQ, KBG * P], f16,
                                                 tag="st1")
                            src = st_d[j, mq * MBQ * P:(mq + 1) * MBQ * P,
                                       k0:k0 + KBG * P]
                            src = src.rearrange("(g p) k -> p g k", p=P)
                            nc.gpsimd.dma_start(st_t[:], src)
                            per_j.append(st_t)
                        st_ts.append(per_j)
                    return st_ts

                # critical-path DMAs first: the first big burst contracts
                # over every xs tile, so xs must land before st group 0.
                xs_tiles = []
                for mb in range(NMB):
                    t = xs_pool.tile([P, CB], f16, tag="xs")
                    nc.sync.dma_start(t[:], xs_d[mb * P:(mb + 1) * P, :])
                    xs_tiles.append(t)
                st_next = dma_st_group(0)
                # stT/xinT feed only the DVE rst chain (slack until the
                # junction) — defer their DMA issue to group 2.
                stT_tiles = [stT_pool.tile([OUT, N], f16, tag="stT",
                                           name=f"stT{i}")
                             for i in range(BC)]

                def dma_stT():
                    for i in range(BC):
                        nc.gpsimd.dma_start(stT_tiles[i][:], stT_d[i])
                        nc.gpsimd.dma_start(rsT_tiles[i][OUT:F, :], xinT_d[i])

                agg_sb = {}        # (g, j, kb) -> agg tile
                aggT_sb = {}       # (i,) -> [tp0_copy, tp1_copy] tiles

                def emit_big_burst(g, j, kb, st_ts):
                    ps0 = agg_ps.tile([P, HALF], f32, tag="aggps",
                                      name=f"ps{g}_{j}_{kb}_0")
                    ps1 = agg_ps.tile([P, HALF], f32, tag="aggps",
                                      name=f"ps{g}_{j}_{kb}_1")
                    for mb in range(NMB):
                        mq, ml = divmod(mb, MBQ)
                        lhsT = st_ts[j][mq][:, ml, kb * P:(kb + 1) * P]
                        nc.tensor.matmul(ps0[:], lhsT,
                                         xs_tiles[mb][:, 0:HALF],
                                         start=(mb == 0), stop=(mb == NMB - 1))
                        nc.tensor.matmul(ps1[:], lhsT,
                                         xs_tiles[mb][:, HALF:CB],
                                         start=(mb == 0), stop=(mb == NMB - 1))
                    t = agg_pool.tile([P, CB], f16, tag="agg",
                                      name=f"agg{g}_{j}_{kb}")
                    drain(t[:, 0:HALF], ps0[:])
                    drain(t[:, HALF:CB], ps1[:])
                    agg_sb[(g, j, kb)] = t

                def emit_tp(g, i):
                    # transpose agg[g] batch-i slices into aggT (f-major)
                    tps = []
                    for jj in range(2):          # j pairs
                        tp = tp_ps.tile([F, 520], f16, tag="tpps",
                                        name=f"tp{g}_{i}_{jj}")
                        n_mm = 0
                        for dj in range(2):
                            j = jj * 2 + dj
                            for kb in range(KBG):
                                c0 = (dj * 2 + kb) * P
                                nc.tensor.matmul(
                                    tp[:, c0:c0 + P],
                                    agg_sb[(g, j, kb)][:, i * F:(i + 1) * F],
                                    ident[:],
                                    start=(n_mm == 0), stop=(n_mm == 3),
                                    is_transpose=True)
                                n_mm += 1
                        tps.append(tp)
                    a0 = aggT_pool.tile([F, 512], f16, tag="aggT",
                                        name=f"aggT{i}_0")
                    a1 = aggT_pool.tile([F, 512], f16, tag="aggT",
                                        name=f"aggT{i}_1")
                    drain(a0[:], tps[0][:, 0:512])
                    drain(a1[:], tps[1][:, 0:512])
                    aggT_sb[i] = (a0, a1)

                def emit_finish(g, i):
                    # project + activate + r*state for batch i of group g
                    k0 = g * KBG * P
                    pp = pp_ps.tile([RU, 260], f32, tag="ppps",
                                    name=f"pp{g}_{i}")
                    for j in range(J):
                        jj, dj = divmod(j, 2)
                        rhs = aggT_sb[i][jj][:, dj * 256:(dj + 1) * 256]
                        nc.tensor.matmul(pp[:, 0:256], wru_t[j][:], rhs,
                                         start=(j == 0), stop=(j == J - 1))
                    nc.scalar.activation(
                        ruT_tiles[i][:, k0:k0 + 256], pp[:, 0:256],
                        AF.Sigmoid, bias=bru_t[:, 0:1])
                    nc.vector.tensor_mul(
                        rsT_tiles[i][0:OUT, k0:k0 + 256],
                        ruT_tiles[i][0:OUT, k0:k0 + 256],
                        stT_tiles[i][:, k0:k0 + 256])

                def chain_step(g, s):
                    # s in [0..8]: step s of the pipelined chain for group g
                    if g < 0:
                        return
                    if s < BC:
                        emit_tp(g, s)
                    if s >= 1:
                        emit_finish(g, s - 1)

                for g in range(G):
                    st_cur = st_next
                    if g + 1 < G:
                        st_next = dma_st_group(g + 1)
                    if g == 0:
                        dma_stT()
                    b = 0
                    for j in range(J):
                        for kb in range(KBG):
                            emit_big_burst(g, j, kb, st_cur)
                            chain_step(g - 1, b)
                            b += 1
                    chain_step(g - 1, 8)
                # tail: chain for last group
                for s in range(BC + 1):
                    chain_step(G - 1, s)

            # ---------------- phase 2 ----------------
            with tc.tile_pool(name="y", bufs=NMB) as y_pool, \
                 tc.tile_pool(name="st2", bufs=10) as st2_pool, \
                 tc.tile_pool(name="unat", bufs=3) as unat_pool, \
                 tc.tile_pool(name="stN", bufs=NMB) as stN_pool, \
                 tc.tile_pool(name="csb", bufs=3) as csb_pool, \
                 tc.tile_pool(name="tmp", bufs=4) as tmp_pool, \
                 tc.tile_pool(name="ost", bufs=3) as ost_pool, \
                 tc.tile_pool(name="utpps", bufs=2, space="PSUM") as utp_ps:

                stN_tiles = []
                for kb in range(NMB):
                    t = stN_pool.tile([P, NAT], f16, tag="stN")
                    nc.gpsimd.dma_start(t[:], stN_d[kb * P:(kb + 1) * P, :])
                    stN_tiles.append(t)

                y_tiles = [y_pool.tile([P, J * NAT], f16, tag="y",
                                       name=f"y{mb}")
                           for mb in range(NMB)]

                def yv_drain(mb, i):
                    v = y_tiles[mb][:].rearrange("p (j i o) -> p j i o",
                                                 j=J, i=BC)
                    return v[:, :, i, :]

                def yv_big(mb, j):
                    v = y_tiles[mb][:].rearrange("p (j io) -> p j io", j=J)
                    return v[:, j, :]

                u_nat = [unat_pool.tile([P, NAT], f16, tag="unat",
                                        name=f"unat{x}") for x in range(3)]

                def emit_utp(i, kb):
                    up = utp_ps.tile([P, OUT], f16, tag="utpps",
                                     name=f"utp{kb}_{i}")
                    nc.tensor.matmul(
                        up[:], ruT_tiles[i][OUT:RU, kb * P:(kb + 1) * P],
                        ident64[OUT:P, :], is_transpose=True)
                    drain(u_nat[kb % 3][:, i * OUT:(i + 1) * OUT], up[:])

                # small-y window (+ u transposes for kb 0 and 1)
                with tc.tile_pool(name="yps", bufs=4, space="PSUM") as y_ps:
                    utp_pre = [(i, kb) for kb in range(2) for i in range(BC)]
                    un = 0
                    for i in range(BC):
                        for mb in range(NMB):
                            yp = y_ps.tile([P, 260], f32, tag="yps",
                                           name=f"yps{i}_{mb}")
                            nc.tensor.matmul(
                                yp[:, 0:YW],
                                rsT_tiles[i][:, mb * P:(mb + 1) * P],
                                wcp_t[:])
                            drain(yv_drain(mb, i),
                                  yp[:, 0:YW].rearrange("p (j o) -> p j o",
                                                        o=OUT))
                            if mb % 8 == 7 and un < len(utp_pre):
                                emit_utp(*utp_pre[un])
                                un += 1

                def dma_st2(kb):
                    ts = []
                    for j in range(J):
                        t = st2_pool.tile([P, NMB, P], f16, tag="st2")
                        src = st_d[j, :, kb * P:(kb + 1) * P]
                        src = src.rearrange("(q p) k -> p q k", p=P)
                        nc.gpsimd.dma_start(t[:], src)
                        ts.append(t)
                    return ts

                with tc.tile_pool(name="cps", bufs=3, space="PSUM") as c_ps:
                    _phase2_bigs(tc, nc, dma_st2, c_ps, csb_pool, tmp_pool,
                                 ost_pool, stN_tiles, u_nat, emit_utp,
                                 yv_big, ones1, bcn_t, out_d, AF, f16, f32,
                                 bc_zero, ones_nat, mybir)

    nc.compile()
    return nc


def _phase2_bigs(tc, nc, dma_st2, c_ps, csb_pool, tmp_pool, ost_pool,
                 stN_tiles, u_nat, emit_utp, yv_big, ones1, bcn_t, out_d,
                 AF, f16, f32, bc_zero, ones_nat, mybir):
                st2_next = dma_st2(0)
                st2_next2 = dma_st2(1)
                for kb in range(NMB):
                    st2_cur = st2_next
                    st2_next = st2_next2
                    st2_next2 = dma_st2(kb + 2) if kb + 2 < NMB else None

                    cp = c_ps.tile([P, NAT], f32, tag="cps", name=f"cps{kb}")
                    for j in range(J):
                        for mb in range(NMB):
                            nc.tensor.matmul(
                                cp[:], st2_cur[j][:, mb, :], yv_big(mb, j),
                                start=(j == 0 and mb == 0),
                                stop=(bc_zero and j == J - 1
                                      and mb == NMB - 1))
                        # sprinkle u-transposes for kb+2
                        if kb + 2 < NMB and j < 4:
                            for di in range(2):
                                emit_utp(j * 2 + di, kb + 2)
                    if not bc_zero:
                        nc.tensor.matmul(cp[:], ones1[:], bcn_t[:],
                                         start=False, stop=True)

                    # last block: split the drain chain in half so the
                    # act/combine/DMA tail after the final matmul shrinks
                    halves = ((0, NAT),) if kb < NMB - 1 else \
                        ((0, NAT // 2), (NAT // 2, NAT))
                    c_sb = csb_pool.tile([P, NAT], f16, tag="csb")
                    t3 = ost_pool.tile([P, NAT], f16, tag="ost")
                    for (c0, c1) in halves:
                        nc.scalar.activation(c_sb[:, c0:c1], cp[:, c0:c1],
                                             AF.Tanh)
                        w = c1 - c0
                        t1 = tmp_pool.tile([P, NAT], f16, tag="tmp")
                        nc.vector.tensor_sub(t1[:, 0:w],
                                             stN_tiles[kb][:, c0:c1],
                                             c_sb[:, c0:c1])
                        t2 = tmp_pool.tile([P, NAT], f16, tag="tmp")
                        nc.vector.tensor_mul(t2[:, 0:w],
                                             u_nat[kb % 3][:, c0:c1],
                                             t1[:, 0:w])
                        nc.vector.tensor_add(t3[:, c0:c1], c_sb[:, c0:c1],
                                             t2[:, 0:w])
                        nc.sync.dma_start(out_d[kb * P:(kb + 1) * P, c0:c1],
                                          t3[:, c0:c1])


def _get_module(bc_zero=False):
    key = f"nc{int(bc_zero)}"
    if key not in _CACHE:
        _CACHE[key] = _build_module(bc_zero)
    return _CACHE[key]


def kernel(input, state, supports, Wr, br, Wu, bu, Wc, bc):
    input = np.asarray(input, np.float32)
    state = np.asarray(state, np.float32)
    supports = np.asarray(supports, np.float32)
    Wr = np.asarray(Wr, np.float32)
    br = np.asarray(br, np.float32)
    Wu = np.asarray(Wu, np.float32)
    bu = np.asarray(bu, np.float32)
    Wc = np.asarray(Wc, np.float32)
    bc = np.asarray(bc, np.float32)

    from concourse.bass_utils import run_bass_kernel_spmd

    nc = _get_module(bc_zero=bool(np.all(bc == 0.0)))
    _CACHE["nc"] = nc

    st_host = np.ascontiguousarray(supports.transpose(0, 2, 1).astype(np.float16))
    wru = np.ascontiguousarray(
        np.concatenate([Wr, Wu], axis=2).astype(np.float16))
    # Wc with feature rows permuted [state(2:66); input(0:2)], flattened (j,o)
    wcp = np.ascontiguousarray(
        np.concatenate([Wc[:, IN:F, :], Wc[:, 0:IN, :]], axis=1)
        .transpose(1, 0, 2).reshape(F, YW).astype(np.float16))
    bru = np.concatenate([br, bu]).reshape(RU, 1).astype(np.float32)
    bcn = np.tile(bc, BC).reshape(1, NAT).astype(np.float16)
    xs_full = np.concatenate([input, state], axis=2)  # [B, N, F]

    in_maps = []
    for c in range(NCORES):
        sl = slice(c * BC, (c + 1) * BC)
        xs_c = np.ascontiguousarray(
            xs_full[sl].transpose(1, 0, 2).reshape(N, CB).astype(np.float16))
        xinT_c = np.ascontiguousarray(
            input[sl].transpose(0, 2, 1).astype(np.float16))
        stT_c = np.ascontiguousarray(
            state[sl].transpose(0, 2, 1).astype(np.float16))
        stN_c = np.ascontiguousarray(
            state[sl].transpose(1, 0, 2).reshape(N, NAT).astype(np.float16))
        in_maps.append({
            "st": st_host,
            "xs": xs_c,
            "xinT": xinT_c,
            "stT": stT_c,
            "stN": stN_c,
            "wru": wru,
            "wcp": wcp,
            "bru": bru,
            "bcn": bcn,
        })

    import time
    t0 = time.monotonic()
    res = run_bass_kernel_spmd(nc, in_maps, core_ids=list(range(NCORES)))
    _CACHE["last_wall_s"] = time.monotonic() - t0

    out = np.empty((B, N, OUT), np.float32)
    for c in range(NCORES):
        o = res.results[c]["out"]               # [N, NAT]
        out[c * BC:(c + 1) * BC] = o.reshape(N, BC, OUT).transpose(1, 0, 2)
    return out
